# revision 51
# baseline (speedup 1.0000x reference)
"""Trainium2 Bass kernel for the nn_Attention sparse-attention module.

Reference computation (per batch b):
  qkv = x @ W_attn + b_attn            [T, 3F]
  q,k,v split -> per head h: [T, D] (D=64, H=16 heads)
  sT[e,d]  = sum_t k[t,e] q[t,d]                (score^T, contract over T)
  s_masked = where(tril, s/sqrt(D), -1e4)       (tril over [D,D])
  w[t,d]   = sum_e s_masked[d,e] v[t,e] / D^2
  w        = softmax(w + mask, axis=t)
  a        = w * v  (elementwise)
  out      = merge(a) @ W_proj + b_proj ; also returns merge(w)

Distribution: data-parallel over B across 8 NeuronCores (2 batches/core).

Per-core schedule (phases; A=qk proj, B=v proj, C=scores+softmax, D=out
proj; suffix = batch index):
    warmup, A0, B0(ev0-3), C0(+A1 and ev4-7 interleaved), D0, B1, C1, D1
The A1 interleave keeps the PE busy during batch 0's softmax chains; the
warmup matmul chain runs during the initial DMA window so the PE clock is
at full p-state when real work arrives.

Precision: the softmax input w is dominated by -10000/D^2 * suffix-sum(v),
so v and its projection (and the w matmul) are f32r; q/k and their
projection are fp8 (score contribution to w is ~1e-2 scale); a=w*v and
W_proj are bf16 (out_a tolerance is ~2e-2, bf16 gives ~3e-3); softmax and
elementwise math stay fp32.
"""

import os
from contextlib import ExitStack

import numpy as np

import concourse.bacc as bacc
import concourse.bass as bass
import concourse.tile as tile
from concourse import mybir
from concourse.bass_utils import run_bass_kernel_spmd

B, T, F, H = 16, 1024, 1024, 16
D = F // H              # 64
NCORES = 8
BPC = B // NCORES       # 2 batches per core
P = 128
KT = F // P             # 8 k-tiles over the feature dim
TBLK = T // P           # 8 t-blocks per batch
HP = H // 2             # 8 head pairs (2 heads stacked on 128 partitions)
NQ = 2 * F // 512       # 4 column chunks of the q|k projection
NG = KT // 2            # 4 fp8 DoubleRow groups (256 contraction rows each)
NWARM = 32              # PE warmup matmuls during the startup DMA window

f32 = mybir.dt.float32
f32r = mybir.dt.float32r
bf16 = mybir.dt.bfloat16
f8 = mybir.dt.float8e4

FAST = True  # kept for test.py compat; only the fast path exists now

_AX = mybir.AxisListType.X
_ADD = mybir.AluOpType.add
_MULT = mybir.AluOpType.mult
_DR = mybir.MatmulPerfMode.DoubleRow


def _build(qk_bias_nz: bool, mask_nz: bool):
    nc = bacc.Bacc("TRN2", target_bir_lowering=False, debug=False)

    xT = nc.dram_tensor("xT", [BPC, F, T], f32r, kind="ExternalInput").ap()
    xTb = nc.dram_tensor("xTb", [BPC, NG, P, 2, T], f8, kind="ExternalInput").ap()
    wqk = nc.dram_tensor(
        "wqk", [NQ, NG, P, 2, 512], f8, kind="ExternalInput"
    ).ap()
    wv = nc.dram_tensor("wv", [KT, P, KT, P], f32r, kind="ExternalInput").ap()
    wp = nc.dram_tensor("wp", [F, F], bf16, kind="ExternalInput").ap()
    bv = nc.dram_tensor("bv", [F], f32, kind="ExternalInput").ap()
    trilm = nc.dram_tensor("trilm", [P, F], f32, kind="ExternalInput").ap()
    trila = nc.dram_tensor("trila", [P, F], f32, kind="ExternalInput").ap()
    bqk = maskd = None
    if qk_bias_nz:
        bqk = nc.dram_tensor("bqk", [2 * F], f32, kind="ExternalInput").ap()
    if mask_nz:
        maskd = nc.dram_tensor("maskd", [BPC, T], f32, kind="ExternalInput").ap()
    out_a = nc.dram_tensor("out_a", [BPC, T, F], f32, kind="ExternalOutput").ap()
    out_w = nc.dram_tensor("out_w", [BPC, F, T], f32, kind="ExternalOutput").ap()


    with tile.TileContext(nc) as tc, ExitStack() as ctx:
        const = ctx.enter_context(tc.tile_pool(name="const", bufs=1))
        xbp = ctx.enter_context(tc.tile_pool(name="xbp", bufs=NG))
        xpool = ctx.enter_context(tc.tile_pool(name="xp", bufs=KT))
        qkp = ctx.enter_context(tc.tile_pool(name="qkp", bufs=2 * NG))
        vpool = ctx.enter_context(tc.tile_pool(name="vp", bufs=4))
        atp = ctx.enter_context(tc.tile_pool(name="atp", bufs=KT))
        wqkp = ctx.enter_context(tc.tile_pool(name="wqkp", bufs=NQ * NG))
        wvp = ctx.enter_context(tc.tile_pool(name="wvp", bufs=KT))
        wpp = ctx.enter_context(tc.tile_pool(name="wpp", bufs=2 * KT))
        wkp = ctx.enter_context(tc.tile_pool(name="wkp", bufs=3))
        sp = ctx.enter_context(tc.tile_pool(name="sp", bufs=KT))
        statp = ctx.enter_context(tc.tile_pool(name="statp", bufs=3))
        outp = ctx.enter_context(tc.tile_pool(name="outp", bufs=4))
        maskp = (
            ctx.enter_context(tc.tile_pool(name="maskp", bufs=2)) if mask_nz else None
        )

        psA = ctx.enter_context(tc.tile_pool(name="psA", bufs=4, space="PSUM"))
        psS = ctx.enter_context(tc.tile_pool(name="psS", bufs=2, space="PSUM"))
        psW = ctx.enter_context(tc.tile_pool(name="psW", bufs=2, space="PSUM"))

        # --- PE warmup: run the clock up to full p-state during the
        # initial DMA window (matmuls on a zeroed tile, no data deps) ---
        zt = const.tile([P, 512], bf16, name="warm")
        nc.vector.memset(zt[:], 0.0)
        for i in range(NWARM):
            wps_ = psA.tile([P, 512], f32, tag="mm", name=f"warm{i}")
            nc.tensor.matmul(wps_[:], zt[:, :P], zt[:], start=True, stop=True)

        # --- prologue DMAs, in priority order ---
        # startup critical: batch-0 fp8 x interleaved with the nq=0 wqk chunk
        xb_t = {}
        wqk_t = {}
        for g in range(NG):
            tb_ = xbp.tile([P, 2, T], f8, tag="xb", name=f"xb0_{g}")
            nc.sync.dma_start(out=tb_[:], in_=xTb[0, g])
            xb_t[(0, g)] = tb_
            w_ = wqkp.tile([P, 2, 512], f8, tag="wqk", name=f"wqk0_{g}")
            nc.sync.dma_start(out=w_[:], in_=wqk[0, g])
            wqk_t[(0, g)] = w_
        for nq in range(1, NQ):
            for g in range(NG):
                w_ = wqkp.tile([P, 2, 512], f8, tag="wqk", name=f"wqk{nq}_{g}")
                nc.sync.dma_start(out=w_[:], in_=wqk[nq, g])
                wqk_t[(nq, g)] = w_
        # batch-1 fp8 x (for the A1 groups that fill the pre-B0 DMA window)
        for g in range(NG):
            tb_ = xbp.tile([P, 2, T], f8, tag="xb", name=f"xb1_{g}")
            nc.sync.dma_start(out=tb_[:], in_=xTb[1, g])
            xb_t[(1, g)] = tb_
        # batch-0 f32r x (stage-2 moving operand)
        x_sb = {}
        for kf in range(KT):
            t_ = xpool.tile([P, T], f32r, tag="x", name=f"x0_{kf}")
            nc.sync.dma_start(out=t_[:], in_=xT[0, kf * P : (kf + 1) * P, :])
            x_sb[(0, kf)] = t_
        # constants (needed by the early score-mask ops)
        bv_t = const.tile([P, KT], f32)
        nc.sync.dma_start(out=bv_t[:], in_=bv.rearrange("(ev p) -> p ev", p=P))
        trilm_t = const.tile([P, F], f32)
        nc.sync.dma_start(out=trilm_t[:], in_=trilm[:])
        trila_t = const.tile([P, F], f32)
        nc.sync.dma_start(out=trila_t[:], in_=trila[:])
        if qk_bias_nz:
            qkb_t = const.tile([P, 2 * F], f32)
            nc.sync.dma_start(out=qkb_t[:], in_=bqk.partition_broadcast(P))
        mask_t = {}
        if mask_nz:
            for b in range(BPC):
                m_ = maskp.tile([P, T], f32, tag="mask", name=f"mask{b}")
                nc.sync.dma_start(out=m_[:], in_=maskd[b].partition_broadcast(P))
                mask_t[b] = m_
        # wv, persistent across both batches
        wv_t = []
        for ev in range(KT):
            w_ = wvp.tile([P, KT, P], f32r, tag="wv", name=f"wvt{ev}")
            nc.sync.dma_start(out=w_[:], in_=wv[ev])
            wv_t.append(w_)
        # W_proj (bf16), persistent
        wp_t = {}
        for nn in range(2):
            for kf in range(KT):
                w_ = wpp.tile([P, 512], bf16, tag="wp", name=f"wp{nn}_{kf}")
                nc.sync.dma_start(
                    out=w_[:],
                    in_=wp[kf * P : (kf + 1) * P, nn * 512 : (nn + 1) * 512],
                )
                wp_t[(nn, kf)] = w_

        # qk storage: per batch, 4 DoubleRow group tiles [P, 2, 2F] fp8
        # holding q|k rows t = g2*256 + j*128 + p.
        qk_sb = {
            b: [
                qkp.tile([P, 2, 2 * F], f8, tag="qk", name=f"qk{b}_{g2}")
                for g2 in range(NG)
            ]
            for b in range(BPC)
        }

        # --- emission helpers ---
        def emit_s1_group(b, nq, tb, copy_eng=None):
            """qk projection: one [128t, 512qkcol] psum group (4 DR accums)."""
            ps = psA.tile([P, 512], f32, tag="mm")
            for g in range(NG):
                nc.tensor.matmul(
                    ps[:],
                    xb_t[(b, g)][:, :, tb * P : (tb + 1) * P],
                    wqk_t[(nq, g)][:],
                    start=(g == 0),
                    stop=(g == NG - 1),
                    perf_mode=_DR,
                )
            g2, j = tb // 2, tb % 2
            dst = qk_sb[b][g2][:, j, nq * 512 : (nq + 1) * 512]
            if qk_bias_nz:
                nc.vector.tensor_tensor(
                    dst, ps[:], qkb_t[:, nq * 512 : (nq + 1) * 512], op=_ADD
                )
            elif copy_eng is None:
                nc.scalar.copy(dst, ps[:])
            else:
                copy_eng.tensor_copy(dst, ps[:])

        _warm_i = [0]

        def emit_warm(n):
            """Dummy matmuls on the zeroed tile: keep the PE clock from
            dropping during known softmax-chain waits (idle => downclock)."""
            for _ in range(n):
                wt_ = psS.tile([P, P], f32, tag="s", name=f"kw{_warm_i[0]}")
                _warm_i[0] += 1
                nc.tensor.matmul(wt_[:], zt[:, :P], zt[:, :P], start=True,
                                 stop=True)

        v_sb = {}

        def emit_v_chunk(b, ev):
            """v projection chunk ev: [128 vfeat, 1024 t] in f32r."""
            vt = vpool.tile([P, T], f32r, tag="v", name=f"v{b}_{ev}")
            pss = [
                psA.tile([P, 512], f32, tag="mm", name=f"vps{b}_{ev}_{i}")
                for i in range(2)
            ]
            for kf in range(KT):
                for tcol in range(2):
                    nc.tensor.matmul(
                        pss[tcol][:],
                        wv_t[ev][:, kf, :],
                        x_sb[(b, kf)][:, tcol * 512 : (tcol + 1) * 512],
                        start=(kf == 0),
                        stop=(kf == KT - 1),
                    )
            for tcol in range(2):
                nc.scalar.activation(
                    vt[:, tcol * 512 : (tcol + 1) * 512],
                    pss[tcol][:],
                    mybir.ActivationFunctionType.Identity,
                    bias=bv_t[:, ev : ev + 1],
                )
            v_sb[(b, ev)] = vt

        a_sb = {}

        def emit_scores(b, hp):
            """Scores for both heads of pair hp: fp8 DR over t, 4 accums.

            k-pair [t,128] x q-pair [t,128] -> [128,128] whose diagonal
            blocks are the two heads' sT; the cross-head blocks land where
            the tril tables multiply by zero.
            """
            sT_ps = psS.tile([P, 2 * D], f32, tag="s", name=f"sps{b}_{hp}")
            for g2 in range(NG):
                nc.tensor.matmul(
                    sT_ps[:],
                    qk_sb[b][g2][:, :, F + hp * 2 * D : F + (hp + 1) * 2 * D],
                    qk_sb[b][g2][:, :, hp * 2 * D : (hp + 1) * 2 * D],
                    start=(g2 == 0),
                    stop=(g2 == NG - 1),
                    perf_mode=_DR,
                )
            # tril mask + scale -> block-diagonal sT_sb [128, 128]
            sT_sb = sp.tile([P, 2 * D], f32r, tag="sT", name=f"sT{b}_{hp}")
            nc.vector.tensor_tensor(
                sT_sb[:], sT_ps[:], trilm_t[:, hp * 2 * D : (hp + 1) * 2 * D],
                op=_MULT,
            )
            nc.vector.tensor_tensor(
                sT_sb[:], sT_sb[:], trila_t[:, hp * 2 * D : (hp + 1) * 2 * D],
                op=_ADD,
            )
            return sT_sb

        def emit_w(b, hp, sT_sb):
            wps = [
                psW.tile([P, 512], f32, tag="w", name=f"wps{b}_{hp}_{tc_}")
                for tc_ in range(2)
            ]
            for tcol in range(2):
                nc.tensor.matmul(
                    wps[tcol][:],
                    sT_sb[:],
                    v_sb[(b, hp)][:, tcol * 512 : (tcol + 1) * 512],
                    start=True,
                    stop=True,
                )
            return wps

        def emit_softmax_a(b, hp, wps):
            """softmax over t (free dim; no max-subtraction needed: |w|<=~64)
            then a = w * v (bf16)."""
            wk = wkp.tile([P, T], f32, tag="wk", name=f"wk{b}_{hp}")
            sums2 = statp.tile([P, 2], f32, tag="sum2", name=f"s2{b}_{hp}")
            sums = statp.tile([P, 1], f32, tag="sum", name=f"sm{b}_{hp}")
            recip = statp.tile([P, 1], f32, tag="rcp", name=f"rc{b}_{hp}")
            for tcol in range(2):
                half = wk[:, tcol * 512 : (tcol + 1) * 512]
                if mask_nz:
                    nc.vector.tensor_tensor(
                        half, wps[tcol][:],
                        mask_t[b][:, tcol * 512 : (tcol + 1) * 512], op=_ADD,
                    )
                    srch = half
                else:
                    srch = wps[tcol][:]
                nc.scalar.activation(
                    half,
                    srch,
                    mybir.ActivationFunctionType.Exp,
                    accum_out=sums2[:, tcol : tcol + 1],
                )
            nc.vector.tensor_reduce(sums[:], sums2[:], axis=_AX, op=_ADD)
            nc.vector.reciprocal(recip[:], sums[:])
            # a = (wk_unnorm * recip) * v on vector (reads pre-norm wk);
            # the in-place normalize for the w output runs on gpsimd.
            at = atp.tile([P, T], bf16, tag="at", name=f"at{b}_{hp}")
            nc.vector.scalar_tensor_tensor(
                at[:], wk[:], recip[:], v_sb[(b, hp)][:], op0=_MULT, op1=_MULT
            )
            nc.scalar.activation(
                wk[:], wk[:], mybir.ActivationFunctionType.Identity,
                scale=recip[:],
            )
            nc.sync.dma_start(out=out_w[b, hp * P : (hp + 1) * P, :], in_=wk[:])
            a_sb[(b, hp)] = at

        def emit_out_partial(b, nn, tb, kf_hi):
            """Accumulate kf in [0, kf_hi) of an out-proj chunk into psum."""
            ps = psA.tile([P, 512], f32, tag="mm")
            for kf in range(kf_hi):
                nc.tensor.matmul(
                    ps[:],
                    a_sb[(b, kf)][:, tb * P : (tb + 1) * P],
                    wp_t[(nn, kf)][:],
                    start=(kf == 0),
                    stop=False,
                )
            return ps

        def emit_out_finish(b, nn, tb, ps, kf_lo):
            for kf in range(kf_lo, KT):
                nc.tensor.matmul(
                    ps[:],
                    a_sb[(b, kf)][:, tb * P : (tb + 1) * P],
                    wp_t[(nn, kf)][:],
                    start=(kf == 0),
                    stop=(kf == KT - 1),
                )
            ot = outp.tile([P, 512], f32, tag="out")
            nc.scalar.copy(ot[:], ps[:])
            nc.sync.dma_start(
                out=out_a[b, tb * P : (tb + 1) * P, nn * 512 : (nn + 1) * 512],
                in_=ot[:],
            )

        def emit_out_chunk(b, nn, tb):
            ps = emit_out_partial(b, nn, tb, 0)
            emit_out_finish(b, nn, tb, ps, 0)

        # --- phase schedule ---
        # A0: batch-0 qk projection
        for nq in range(NQ):
            for tb in range(TBLK):
                emit_s1_group(0, nq, tb)
        # Early scores for batch 0 plus the first A1 groups: PE work that
        # needs no stage-2 inputs, filling the window while xT/wv stream in.
        a1_work = [(nq, tb) for nq in range(NQ) for tb in range(TBLK)]
        a1_i = 0
        sT0 = []
        for hp in range(HP):
            sT0.append(emit_scores(0, hp))
        for _ in range(8):
            emit_s1_group(1, *a1_work[a1_i],
                          copy_eng=nc.vector if a1_i % 2 else None)
            a1_i += 1
        # B0: first 4 v chunks en bloc (rest interleave into C0)
        for ev in range(4):
            emit_v_chunk(0, ev)
        # C0: softmax for batch 0, with A1 (batch-1 qk projection)
        # and the remaining v chunks interleaved to keep the PE fed.
        for hp in range(HP):
            if 2 <= hp <= 5:
                emit_v_chunk(0, hp + 2)
            if hp == 6:
                # batch-1 f32r x: slots free after the last b0 v chunk;
                # lands during D0
                for kf in range(KT):
                    t_ = xpool.tile([P, T], f32r, tag="x", name=f"x1_{kf}")
                    nc.sync.dma_start(
                        out=t_[:], in_=xT[1, kf * P : (kf + 1) * P, :]
                    )
                    x_sb[(1, kf)] = t_
            emit_s1_group(1, *a1_work[a1_i],
                          copy_eng=nc.vector if a1_i % 2 else None)
            a1_i += 1
            wps = emit_w(0, hp, sT0[hp])
            for _ in range(2):
                emit_s1_group(1, *a1_work[a1_i],
                              copy_eng=nc.vector if a1_i % 2 else None)
                a1_i += 1
            emit_softmax_a(0, hp, wps)
        # D0: batch-0 output projection
        for nn in range(2):
            for tb in range(TBLK):
                emit_out_chunk(0, nn, tb)
        # B1: first 5 v chunks; ev 5..7 interleave into C1 as PE filler
        for ev in range(5):
            emit_v_chunk(1, ev)
        # C1: batch-1 scores/softmax; the tail v chunks plus partial
        # out-proj accumulation keep the PE fed through the softmax chains.
        d1_pre = {}
        for hp in range(HP):
            if 3 <= hp <= 5:
                emit_v_chunk(1, hp + 2)
            sT_sb = emit_scores(1, hp)
            wps = emit_w(1, hp, sT_sb)
            if hp == 6:
                for tb in range(4):
                    d1_pre[(0, tb)] = emit_out_partial(1, 0, tb, 6)
            elif hp == 7:
                # extend the prefixed chunks with kf=6 (a[6] is ready)
                for tb in range(4):
                    nc.tensor.matmul(
                        d1_pre[(0, tb)][:],
                        a_sb[(1, 6)][:, tb * P : (tb + 1) * P],
                        wp_t[(0, 6)][:],
                        start=False,
                        stop=False,
                    )
            emit_softmax_a(1, hp, wps)
        # D1: finish the prefixed chunks (only kf=7 outstanding), then rest
        for tb in range(4):
            emit_out_finish(1, 0, tb, d1_pre[(0, tb)], 7)
        for nn in range(2):
            for tb in range(TBLK):
                if nn == 0 and tb < 4:
                    continue
                emit_out_chunk(1, nn, tb)

    nc.compile()
    return nc


_NC_CACHE: dict = {}


def _get_nc(fast: bool, qk_bias_nz: bool, mask_nz: bool):
    key = (qk_bias_nz, mask_nz)
    if key not in _NC_CACHE:
        _NC_CACHE[key] = _build(*key)
    return _NC_CACHE[key]


def _tril_tables(fast=True):
    """Tril scale/offset tables [128, 1024], one 128x64 block per head.

    sT_ps[h2*64+e, d] holds sum_t k[t,e] q[t,d] for head 2*hp+h2.
    sT_sb[:, h2*64+d] = sT_ps_rep * trilm + trila: within the head's own
    e-rows, kept entries (d >= e) scale by 1/(sqrt(D)*D^2) and masked
    entries become -10000/D^2; the other head's rows are zeroed so the
    pair's [128,128] block is block-diagonal and one matmul can contract
    all 128 partitions.
    """
    e = np.arange(D)[:, None]
    d = np.arange(D)[None, :]
    kept = (d >= e)
    qk_scale = 1024.0  # host prescales Wqk by 32 for fp8; q,k each carry x32
    mul_blk = np.where(
        kept, np.float32(1.0 / (8.0 * 4096.0 * qk_scale)), np.float32(0.0)
    )
    add_blk = np.where(kept, np.float32(0.0), np.float32(-10000.0 / 4096.0))
    trilm = np.zeros((P, F), np.float32)
    trila = np.zeros((P, F), np.float32)
    for h in range(H):
        hp, h2 = h // 2, h % 2
        rows = slice(h2 * D, (h2 + 1) * D)
        cols = slice(h * D, (h + 1) * D)
        trilm[rows, cols] = mul_blk
        trila[rows, cols] = add_blk
    return trilm, trila


def _install_ntff_hook_shim():
    """Provide antenv.axon_hooks for trace=True profiling under axon.

    The agent image's antenv package lacks axon_hooks; replicate the
    ctypes-based NTFF hook from the boot script so bass_utils can
    capture per-core NTFF profiles (exec_time_ns).
    """
    import contextlib
    import ctypes
    import sys
    import types

    try:
        from antenv import axon_hooks  # noqa: F401

        return
    except ImportError:
        pass

    hook = None
    try:
        lib = ctypes.CDLL("/opt/axon/libaxon_pjrt.so")
        if hasattr(lib, "axon_start_nrt_profile"):
            lib.axon_start_nrt_profile.argtypes = [
                ctypes.POINTER(ctypes.c_int64),
                ctypes.c_size_t,
            ]
            lib.axon_start_nrt_profile.restype = ctypes.c_int64
            lib.axon_stop_nrt_profile.argtypes = [ctypes.c_char_p]
            lib.axon_stop_nrt_profile.restype = ctypes.c_int64

            @contextlib.contextmanager
            def _hook(output_dir, device_ids):
                import jax

                jax.devices()
                if device_ids:
                    ids = (ctypes.c_int64 * len(device_ids))(*device_ids)
                    rc = lib.axon_start_nrt_profile(ids, len(device_ids))
                else:
                    rc = lib.axon_start_nrt_profile(None, 0)
                if rc != 0:
                    raise RuntimeError(f"axon_start_nrt_profile rc={rc}")
                try:
                    yield
                finally:
                    n = lib.axon_stop_nrt_profile(str(output_dir).encode())
                    print(f"ntff profile: {n} file(s) -> {output_dir}")

            hook = _hook
    except OSError:
        pass

    mod = types.ModuleType("antenv.axon_hooks")
    mod.get_axon_ntff_profile_hook = lambda: hook
    mod.set_axon_ntff_profile_hook = lambda h: None
    sys.modules["antenv.axon_hooks"] = mod


def kernel(x, mask, W_attn, b_attn, W_proj, b_proj, _trace=False):
    if _trace:
        _install_ntff_hook_shim()
    x = np.ascontiguousarray(np.asarray(x, dtype=np.float32))
    mask = np.asarray(mask, dtype=np.float32)
    W_attn = np.ascontiguousarray(np.asarray(W_attn, dtype=np.float32))
    b_attn = np.asarray(b_attn, dtype=np.float32)
    W_proj = np.ascontiguousarray(np.asarray(W_proj, dtype=np.float32))
    b_proj = np.asarray(b_proj, dtype=np.float32)

    qk_bias_nz = bool(np.any(b_attn[: 2 * F]))
    mask_nz = bool(np.any(mask))
    nc = _get_nc(True, qk_bias_nz, mask_nz)

    # host-side layout prep
    xT = np.ascontiguousarray(
        x.reshape(NCORES, BPC, T, F).transpose(0, 1, 3, 2)
    )  # [cores, BPC, F, T]
    mask_c = mask.reshape(B, T).reshape(NCORES, BPC, T)
    import ml_dtypes

    f8np = ml_dtypes.float8_e4m3
    wqk_h = np.ascontiguousarray((W_attn[:, : 2 * F] * 32.0).astype(f8np))
    wv_ = np.ascontiguousarray(W_attn[:, 2 * F :])
    wp_h = np.ascontiguousarray(W_proj.astype(ml_dtypes.bfloat16))
    bv_ = np.ascontiguousarray(b_attn[2 * F :])
    trilm, trila = _tril_tables(True)

    in_maps = []
    for c in range(NCORES):
        m = {
            "xT": xT[c],
            "xTb": xT[c].astype(f8np),
            "wqk": wqk_h,
            "wv": wv_,
            "wp": wp_h,
            "bv": bv_,
            "trilm": trilm,
            "trila": trila,
        }
        if qk_bias_nz:
            m["bqk"] = np.ascontiguousarray(b_attn[: 2 * F])
        if mask_nz:
            m["maskd"] = np.ascontiguousarray(mask_c[c])
        in_maps.append(m)

    kw = {}
    if _trace and os.environ.get("BASS_ATTN_TRACE_DIR"):
        kw["tmpdir"] = os.environ["BASS_ATTN_TRACE_DIR"]
    res = run_bass_kernel_spmd(nc, in_maps, list(range(NCORES)), trace=_trace, **kw)
    kernel._last_exec_ns = res.exec_time_ns
    kernel._last_res = res

    a = np.concatenate([r["out_a"] for r in res.results], axis=0).reshape(B, T, F)
    if np.any(b_proj):
        a = a + b_proj[None, None, :]
    wT = np.concatenate([r["out_w"] for r in res.results], axis=0).reshape(B, F, T)
    w = np.ascontiguousarray(wT.transpose(0, 2, 1))
    return a, w


kernel._last_exec_ns = None
